# revision 24
# baseline (speedup 1.0000x reference)
"""Trainium2 Bass kernel for nn_BaseAttentionEncoder.

Reference computation (per batch element b, shapes L=2048, D=256):
    Q = x @ Wq.T + bq ; K = x @ Wk.T + bk ; V = x @ Wv.T + bv
    E = Q @ K.T / 16
    A = softmax(E, axis=q)            # normalized over the QUERY axis (dim 1)
    C = A @ V
    outputs: S_p = concat([Q, C], axis=rows) [2L, D],  A [L, L]

Strategy: pure data parallelism — one batch element per NeuronCore (B=8 =
n_cores). The small 256x256 projection weights are replicated. Inside a core
everything is laid out so the softmax axis (q) is a free axis:
    E^T[k, q] = K @ Q^T computed with k on partitions -> exp+row-sum fused on
    ScalarE (accum_out), reciprocal + per-partition normalize on VectorE.
    C = sum_k matmul(lhsT=P[k, q-tile], rhs=V[k, :]) needs exactly the
    [k-partition, q-free] layout P already has. The A output needs P
    transposed, done as 256 TensorE 128x128 transposes staged through PSUM.
Matmuls run in fp16 (1 cycle/row on TRN2 vs 4 for fp32); softmax statistics,
normalization and all outputs stay fp32.
"""

import os
import sys
from contextlib import nullcontext

sys.path.insert(0, "/opt/trn_rl_repo")

import numpy as np

import concourse.tile as tile
from concourse import bacc, mybir
from concourse.bass_utils import run_bass_kernel_spmd

F32 = mybir.dt.float32
F16 = mybir.dt.float16
AFT = mybir.ActivationFunctionType

B, L, D = 8, 2048, 256
P = 128
LT = L // P      # 16  l/k/q 128-tiles
DC = D // P      # 2   d/o 128-chunks
N_CORES = 8
SCALE = 1.0 / (D ** 0.5)  # 1/16


def _build(reps: int = 1, timing: bool = False):
    nc = bacc.Bacc()
    x_p = nc.declare_dram_parameter("x", [L, D], F32, isOutput=False)
    wq_p = nc.declare_dram_parameter("wq", [D, D], F32, isOutput=False)
    wk_p = nc.declare_dram_parameter("wk", [D, D], F32, isOutput=False)
    wv_p = nc.declare_dram_parameter("wv", [D, D], F32, isOutput=False)
    bq_p = nc.declare_dram_parameter("bq", [D], F32, isOutput=False)
    bk_p = nc.declare_dram_parameter("bk", [D], F32, isOutput=False)
    bv_p = nc.declare_dram_parameter("bv", [D], F32, isOutput=False)
    id_p = nc.declare_dram_parameter("ident16", [P, P], F16, isOutput=False)
    if timing:
        # full-size internal scratch targets keep the HBM write traffic real
        # while the external output (and thus the host transfer) stays tiny.
        sp_p = nc.dram_tensor("sp_scratch", [2 * L, D], F32)
        a_p = nc.dram_tensor("a_scratch", [L, L], F32)
        done_p = nc.declare_dram_parameter("done", [1, 1], F32, isOutput=True)
    else:
        sp_p = nc.declare_dram_parameter("sp", [2 * L, D], F32, isOutput=True)
        a_p = nc.declare_dram_parameter("a", [L, L], F32, isOutput=True)

    with tile.TileContext(nc) as tc:
        with (
            tc.tile_pool(name="singles", bufs=1) as singles,
            tc.tile_pool(name="big", bufs=1) as big,
            tc.tile_pool(name="astage", bufs=2) as astage,
            tc.tile_pool(name="small_out", bufs=4) as small_out,
            # PSUM: erow [128,1024]f32 = 2 banks x2 bufs, atile [128,2048]f16
            # = 2 banks x2 bufs -> 8 banks exactly.
            tc.tile_pool(name="psA", bufs=2, space="PSUM") as psA,
            tc.tile_pool(name="psB", bufs=2, space="PSUM") as psB,
        ):
          for _rep in range(1 if timing else reps):
            with (tc.For_i(0, reps, 1, hint_engines=(
                    mybir.EngineType.PE, mybir.EngineType.DVE,
                    mybir.EngineType.Activation, mybir.EngineType.SP,
                    mybir.EngineType.Pool)) if timing else nullcontext()):
                # ---------------- phase 0: loads ----------------
                ident = singles.tile([P, P], F16, tag="ident")
                nc.sync.dma_start(out=ident, in_=id_p[:, :])
                ident32 = singles.tile([P, P], F32, tag="ident32")
                nc.vector.tensor_copy(ident32, ident)
                ones_row = singles.tile([1, P], F16, tag="ones")
                nc.vector.memset(ones_row, 1.0)

                # x natural [p, lt, d] fp32 (HWDGE; cast happens on the
                # PSUM->SBUF copy after the transpose)
                xn = big.tile([P, LT, D], F32, tag="xn")
                x_r = x_p[:].rearrange("(lt p) d -> p lt d", p=P)
                for lg in range(4):
                    nc.sync.dma_start(
                        out=xn[:, lg * 4:(lg + 1) * 4, :],
                        in_=x_r[:, lg * 4:(lg + 1) * 4, :],
                    )
                # weights natural [p, oc, d] fp32 (HWDGE)
                wns = {}
                for name, w_p in (("q", wq_p), ("k", wk_p), ("v", wv_p)):
                    wn = big.tile([P, DC, D], F32, tag=f"wn{name}")
                    nc.sync.dma_start(
                        out=wn, in_=w_p[:].rearrange("(oc p) d -> p oc d", p=P)
                    )
                    wns[name] = wn
                # per-partition bias layout [p, oc] fp32 (ScalarE bias operand)
                bq_pp = singles.tile([P, DC], F32, tag="bq_pp")
                bk_pp = singles.tile([P, DC], F32, tag="bk_pp")
                nc.sync.dma_start(out=bq_pp, in_=bq_p[:].rearrange("(oc p) -> p oc", p=P))
                nc.sync.dma_start(out=bk_pp, in_=bk_p[:].rearrange("(oc p) -> p oc", p=P))
                # row-vector biases [1, D]: f32 loads, cast on DVE
                bq_row32 = singles.tile([1, D], F32, tag="bq_row32")
                bv_row32 = singles.tile([1, D], F32, tag="bv_row32")
                nc.sync.dma_start(out=bq_row32, in_=bq_p[:][None, :])
                nc.sync.dma_start(out=bv_row32, in_=bv_p[:][None, :])
                bq_row = singles.tile([1, D], F16, tag="bq_row")
                bv_row = singles.tile([1, D], F16, tag="bv_row")
                nc.vector.tensor_copy(bq_row, bq_row32)
                nc.vector.tensor_copy(bv_row, bv_row32)

                # ---------------- phase 1: transposes of x and W --------------
                # xT [p=d%128, dc, l] ; WT [p=d%128, dc, o]
                xT = big.tile([P, DC, L], F16, tag="xT")
                for lt in range(LT):
                    for dc in range(DC):
                        pt = psB.tile([P, 512], F32, tag="atile")
                        nc.tensor.transpose(
                            pt[:, :P], xn[:, lt, dc * P:(dc + 1) * P], ident32
                        )
                        nc.vector.tensor_copy(
                            xT[:, dc, lt * P:(lt + 1) * P], pt[:, :P]
                        )
                wts = {}
                for name in ("q", "k", "v"):
                    wt = big.tile([P, DC, D], F16, tag=f"wt{name}")
                    for oc in range(DC):
                        for dc in range(DC):
                            pt = psB.tile([P, 512], F32, tag="atile")
                            nc.tensor.transpose(
                                pt[:, :P], wns[name][:, oc, dc * P:(dc + 1) * P],
                                ident32,
                            )
                            nc.vector.tensor_copy(
                                wt[:, dc, oc * P:(oc + 1) * P], pt[:, :P]
                            )
                    wts[name] = wt

                # ---------------- phase 2: projections ----------------
                # Qt/Kt transposed layouts [p=o%128, oc, l] fp16 (E inputs)
                qt_t = big.tile([P, DC, L], F16, tag="qt")
                kt_t = big.tile([P, DC, L], F16, tag="kt")
                for dst, wname, bias_pp in ((qt_t, "q", bq_pp), (kt_t, "k", bk_pp)):
                    for oc in range(DC):
                        for qh in range(2):  # 1024-wide halves of l
                            ps = psA.tile([P, 1024], F32, tag="erow")
                            for h in range(2):
                                for dc in range(DC):
                                    nc.tensor.matmul(
                                        ps[:, h * 512:(h + 1) * 512],
                                        lhsT=wts[wname][:, dc, oc * P:(oc + 1) * P],
                                        rhs=xT[:, dc, qh * 1024 + h * 512:
                                               qh * 1024 + (h + 1) * 512],
                                        start=(dc == 0),
                                        stop=(dc == DC - 1),
                                    )
                            nc.scalar.activation(
                                dst[:, oc, qh * 1024:(qh + 1) * 1024],
                                ps,
                                AFT.Identity,
                                bias=bias_pp[:, oc:oc + 1],
                            )

                # V natural [p=k%128, kt, d] fp16 (C input), bias via rank-1
                vn = big.tile([P, LT, D], F16, tag="vn")
                # Q natural: per-lt fp32 tiles, DMA'd straight to S_p rows 0..L
                for wname, brow, odt in (("v", bv_row, F16), ("q", bq_row, F32)):
                    for lt in range(LT):
                        ps = psA.tile([P, 1024], F32, tag="erow")
                        for dc in range(DC):
                            nc.tensor.matmul(
                                ps[:, :D],
                                lhsT=xT[:, dc, lt * P:(lt + 1) * P],
                                rhs=wts[wname][:, dc, :],
                                start=(dc == 0),
                                stop=False,
                            )
                        nc.tensor.matmul(
                            ps[:, :D],
                            lhsT=ones_row,
                            rhs=brow,
                            start=False,
                            stop=True,
                        )
                        if odt == F16:
                            nc.vector.tensor_copy(vn[:, lt, :], ps[:, :D])
                        else:
                            qnt = small_out.tile([P, D], F32, tag="qnt")
                            nc.scalar.activation(qnt, ps[:, :D], AFT.Copy)
                            nc.sync.dma_start(
                                out=sp_p[lt * P:(lt + 1) * P, :], in_=qnt
                            )

                # ---------------- phase 3: E^T rows, exp, softmax stats -------
                # P[p=k%128, kt, q] = exp(E^T/16) then normalized by 1/col-sum
                pmat = big.tile([P, LT, L], F16, tag="pmat")
                sacc = singles.tile([P, LT, 2], F32, tag="sacc")
                ssum = singles.tile([P, LT], F32, tag="ssum")
                rrec = singles.tile([P, LT], F32, tag="rrec")
                GA = 2  # k-tiles per A write-group (1 KB contiguous chunks)
                a_group = [None]
                NO_A = timing and bool(int(os.environ.get("KERNEL_NO_A", "0")))
                NO_C = timing and bool(int(os.environ.get("KERNEL_NO_C", "0")))
                NO_E = timing and bool(int(os.environ.get("KERNEL_NO_E", "0")))

                def emit_a_strip(kt):
                    if NO_A:
                        return
                    # A column block kt*128..+128: transpose pmat[:, kt, :]
                    # one qt-tile at a time, stage as [p, qt, g, j]; DMA out
                    # a 4-kt group at once so each DRAM run is 512 floats.
                    g = kt % GA
                    if g == 0:
                        a_group[0] = astage.tile([P, LT, GA, P], F16, tag="asb", name="asb")
                    asb = a_group[0]
                    pa = psB.tile([P, L], F16, tag="atile")
                    for qt in range(LT):
                        nc.tensor.transpose(
                            pa[:, qt * P:(qt + 1) * P],
                            pmat[:, kt, qt * P:(qt + 1) * P],
                            ident,
                        )
                    nc.vector.tensor_copy(
                        asb[:, :, g, :],
                        pa.rearrange("p (qt j) -> p qt j", j=P),
                    )
                    if g == GA - 1:
                        # SWDGE DMA casts the f16 staging to the f32 output
                        ktg = kt // GA
                        nc.gpsimd.dma_start(
                            out=a_p[:, ktg * GA * P:(ktg + 1) * GA * P].rearrange(
                                "(qt p) c -> p qt c", p=P
                            ),
                            in_=asb.rearrange("p qt g j -> p qt (g j)"),
                        )

                cnh = big.tile([P, LT, D], F32, tag="cnh")

                def emit_c_half1(qt):
                    ps1 = psA.tile([P, 1024], F32, tag="erow")
                    for kt in range(LT // 2):
                        nc.tensor.matmul(
                            ps1[:, :D],
                            lhsT=pmat[:, kt, qt * P:(qt + 1) * P],
                            rhs=vn[:, kt, :],
                            start=(kt == 0),
                            stop=(kt == LT // 2 - 1),
                        )
                    nc.scalar.activation(cnh[:, qt, :], ps1[:, :D], AFT.Copy)

                c1_next = [0]

                def emit_c_half1_some(n):
                    while c1_next[0] < min(n, LT):
                        emit_c_half1(c1_next[0])
                        c1_next[0] += 1

                for kt in range(0 if NO_E else LT):
                    for qh in range(2):
                        ps = psA.tile([P, 1024], F32, tag="erow")
                        for h in range(2):
                            for oc in range(DC):
                                nc.tensor.matmul(
                                    ps[:, h * 512:(h + 1) * 512],
                                    lhsT=kt_t[:, oc, kt * P:(kt + 1) * P],
                                    rhs=qt_t[:, oc, qh * 1024 + h * 512:
                                             qh * 1024 + (h + 1) * 512],
                                    start=(oc == 0),
                                    stop=(oc == DC - 1),
                                )
                        nc.scalar.activation(
                            pmat[:, kt, qh * 1024:(qh + 1) * 1024],
                            ps,
                            AFT.Exp,
                            scale=SCALE,
                            accum_out=sacc[:, kt, qh:qh + 1],
                        )
                    nc.vector.reduce_sum(
                        ssum[:, kt:kt + 1], sacc[:, kt, :], axis=mybir.AxisListType.X
                    )
                    nc.vector.reciprocal(rrec[:, kt:kt + 1], ssum[:, kt:kt + 1])
                    nc.vector.tensor_scalar_mul(
                        pmat[:, kt, :], pmat[:, kt, :], rrec[:, kt:kt + 1]
                    )
                    if kt >= 2:
                        emit_a_strip(kt - 2)
                    if not NO_C and kt >= 9:
                        emit_c_half1_some((kt - 8) * 3)
                if not NO_E:
                    emit_a_strip(LT - 2)
                    emit_a_strip(LT - 1)
                    if not NO_C:
                        emit_c_half1_some(LT)

                # ---------------- phase 4+5: C accumulation and A output ------
                for qt in range(0 if NO_C else LT):
                    psc = psA.tile([P, 1024], F32, tag="erow")
                    for kt in range(LT // 2, LT):
                        nc.tensor.matmul(
                            psc[:, :D],
                            lhsT=pmat[:, kt, qt * P:(qt + 1) * P],
                            rhs=vn[:, kt, :],
                            start=(kt == LT // 2),
                            stop=(kt == LT - 1),
                        )
                    cnq = small_out.tile([P, D], F32, tag="qnt")
                    nc.vector.tensor_add(cnq, psc[:, :D], cnh[:, qt, :])
                    nc.sync.dma_start(
                        out=sp_p[L + qt * P:L + (qt + 1) * P, :], in_=cnq
                    )

            if timing:
                dflag = singles.tile([1, 1], F32, tag="dflag")
                nc.vector.memset(dflag, 1.0)
                nc.sync.dma_start(out=done_p[:, :], in_=dflag)

    nc.finalize()
    return nc


_IDENT16 = np.eye(128, dtype=np.float16)

_NC_CACHE = {}
LAST_RESULT = None  # BassKernelResults of the most recent kernel() call


def kernel(x, Wq, bq, Wk, bk, Wv, bv):
    global LAST_RESULT
    reps = int(os.environ.get("KERNEL_REPS", "1"))
    if reps not in _NC_CACHE:
        _NC_CACHE[reps] = _build(reps)
    nc = _NC_CACHE[reps]

    def f(a):
        return np.ascontiguousarray(np.asarray(a, dtype=np.float32))

    x, Wq, bq, Wk, bk, Wv, bv = map(f, (x, Wq, bq, Wk, bk, Wv, bv))
    in_maps = [
        {
            "x": x[b],
            "wq": Wq, "wk": Wk, "wv": Wv,
            "bq": bq, "bk": bk, "bv": bv,
            "ident16": _IDENT16,
        }
        for b in range(B)
    ]
    trace = bool(int(os.environ.get("KERNEL_TRACE", "0")))
    LAST_RESULT = run_bass_kernel_spmd(
        nc, in_maps, list(range(N_CORES)), trace=trace
    )
    res = LAST_RESULT.results
    S_p = np.stack([res[b]["sp"] for b in range(B)], axis=0)
    A = np.stack([res[b]["a"] for b in range(B)], axis=0)
    return (S_p, A)
